# revision 1
# baseline (speedup 1.0000x reference)
"""Trainium2 Bass kernel: multi-head attention forward (B=2, S=2048, D=1024, H=16).

Sharding: 8 cores = data-parallel over batch (2) x tensor-parallel over heads
(4 head-groups of 4 heads).  Each core computes, for its batch b and head
group g:

    q/k/v projections for its 4 heads (column-sharded Wq/Wk/Wv),
    causal-softmax attention for those heads,
    a partial output projection (row-sharded Wo).

Host side: inputs are pre-transposed/sliced per core; the 4 partial outputs
per batch are summed and the bias added on the host (the "unshard").

Per-core device algorithm (all matmuls in float32r: 1 cycle/row on the PE):
  qT = wq.T @ x.T   [256, S]   (weights stationary)
  kT = wk.T @ x.T   [256, S]
  v  = x @ wv       [S, 256]   (x.T chunks stationary)
  per head h, per 1024-wide query block j:
      scoresT[sk, sq] = k_h @ q_h.T     (K=64 matmuls, psum)
      attnT = exp(scale * scoresT)      (ACT, psum -> SBUF; no max-sub:
                                         |scale*scores| is small so exp is safe)
      causal mask on diagonal tiles     (GPSIMD affine_select, fill 0)
      ctxT'[65, sq] = [v_h | 1] PV matmul with a ones column ->
                      row 64 = softmax denominators
      recip = 1/d                       (DVE custom op, ~2 ULP; SBUF src only)
      ctx = ctxT'[0:64] * bcast(recip)  (GPSIMD partition_broadcast + DVE mul)
  out_partial = ctx-stacked @ wo        (summed across cores on host)

"""

import sys

sys.path.insert(0, "/opt/trn_rl_repo")

import numpy as np

B, S, D = 2, 2048, 1024
H = 16
DH = 64
HL = 4  # heads per core
NCORES = 8

_PROGRAM_CACHE = {}


def build_program(S=S, D=D, HL=HL, DH=DH, debug_dumps=()):
    import concourse.tile as tile
    from concourse import bacc, mybir

    f32 = mybir.dt.float32
    f32r = mybir.dt.float32r
    A = mybir.ActivationFunctionType
    Alu = mybir.AluOpType

    KD = D // 128        # contraction chunks for the projections
    M = HL * DH          # per-core projected width (256)
    MQ = M // 128        # qT/kT partition tiles (2)
    ST = S // 128        # 128-row s tiles
    W = min(1024, S)     # query-block width
    NJ = S // W          # query blocks
    CW = W // 512        # 512-chunks per query block
    TPB = W // 128       # 128-tiles of sk per query block
    scale = 1.0 / float(np.sqrt(DH))

    nc = bacc.Bacc("TRN2", target_bir_lowering=False, debug=False)
    xT = nc.dram_tensor("xT", (D, S), f32r, kind="ExternalInput").ap()
    wq = nc.dram_tensor("wq", (D, M), f32r, kind="ExternalInput").ap()
    wk = nc.dram_tensor("wk", (D, M), f32r, kind="ExternalInput").ap()
    wv = nc.dram_tensor("wv", (D, M), f32r, kind="ExternalInput").ap()
    wo = nc.dram_tensor("wo", (M, D), f32r, kind="ExternalInput").ap()
    out = nc.dram_tensor("out", (S, D), f32, kind="ExternalOutput").ap()
    if "norm" in debug_dumps:
        NJd = NJ
        dr = nc.dram_tensor("dr", (HL, NJd, W), f32, kind="ExternalOutput").ap()
        dbc = nc.dram_tensor("dbc", (HL, NJd, 64, W), f32,
                             kind="ExternalOutput").ap()
        dcu = nc.dram_tensor("dcu", (HL, NJd, DH + 1, W), f32,
                             kind="ExternalOutput").ap()

    with tile.TileContext(nc) as tc:
        with (
            tc.tile_pool(name="weights", bufs=1) as wpool,
            tc.tile_pool(name="persist", bufs=1) as mpool,
            tc.tile_pool(name="attn", bufs=7) as apool,
            tc.tile_pool(name="rp", bufs=2) as rpool,
        ):
            wo_sb = wpool.tile([128, MQ, D], f32r, tag="wo")
            qT_sb = mpool.tile([128, MQ, S], f32r, tag="qT")
            kT_sb = mpool.tile([128, MQ, S], f32r, tag="kT")
            v_sb = mpool.tile([128, ST, HL * (DH + 1)], f32r, tag="v")
            ctx_sb = mpool.tile([128, MQ, S], f32r, tag="ctx")

            # ---------- output-projection step (one 128-row s tile) --------
            ostage = [None]

            def emit_outproj_st(st, pool, tag):
                ops_t = pool.tile([128, D], f32, tag=tag, name="ops_t")
                for n in range(D // 512):
                    for p2 in range(MQ):
                        nc.tensor.matmul(
                            ops_t[:, n * 512:(n + 1) * 512],
                            ctx_sb[:, p2, st * 128:(st + 1) * 128],
                            wo_sb[:, p2, n * 512:(n + 1) * 512],
                            start=(p2 == 0),
                            stop=(p2 == MQ - 1),
                        )
                o_sb = ostage[0].tile([128, D], f32, tag="o")
                nc.vector.tensor_copy(o_sb[:], ops_t[:])
                nc.sync.dma_start(out[st * 128:(st + 1) * 128, :], o_sb[:])

            # ---------- attention step machinery (emitted lazily) ----------
            def attention_hj(h, j, companion=None):
                """Generator: one yield per sk-tile step; tail does the
                PV flush and softmax normalization."""
                hm, po = h // 2, 64 * (h % 2)
                qrow = slice(po, po + DH)
                nski = TPB * (j + 1)
                ctx_ps = cpool.tile([DH + 1, W], f32, tag="ctx_ps")
                pending = []

                def emit_pv(item):
                    ski, attn_t, c0, ex0 = item
                    for n in range(c0, CW):
                        lo = ex0 if n == c0 else n * 512
                        nc.tensor.matmul(
                            ctx_ps[:, lo:(n + 1) * 512],
                            v_sb[:, ski, h * (DH + 1):(h + 1) * (DH + 1)],
                            attn_t[:, lo:(n + 1) * 512],
                            start=(ski == 0),
                            stop=(ski == min(nski - 1, TPB * j + 4 * n + 3)),
                        )

                for ski in range(nski):
                    c0 = max(0, ski // 4 - CW * j)
                    off = c0 * 512
                    # columns below the diagonal start are fully masked;
                    # scores/exp/PV all skip them
                    ex0 = max(off, 128 * ski - j * W) if ski >= TPB * j else off
                    sc_ps = spool.tile([128, W], f32, tag="big")
                    for n in range(c0, CW):
                        lo = ex0 if n == c0 else n * 512
                        nc.tensor.matmul(
                            sc_ps[:, lo:(n + 1) * 512],
                            kT_sb[qrow, hm, ski * 128:(ski + 1) * 128],
                            qT_sb[qrow, hm, j * W + lo: j * W + (n + 1) * 512],
                            start=True,
                            stop=True,
                        )
                    attn_t = apool.tile([128, W], f32r, tag="attn")
                    nc.scalar.activation(
                        attn_t[:, ex0:W], sc_ps[:, ex0:W], A.Exp, scale=scale
                    )
                    if ski >= TPB * j:  # diagonal-crossing tile
                        cross_end = 128 * ski + 128 - j * W
                        nc.gpsimd.affine_select(
                            out=attn_t[:, ex0:cross_end],
                            in_=attn_t[:, ex0:cross_end],
                            compare_op=Alu.is_ge,
                            fill=0.0,
                            base=j * W + ex0 - 128 * ski,
                            pattern=[[1, cross_end - ex0]],
                            channel_multiplier=-1,
                        )
                    pending.append((ski, attn_t, c0, ex0))
                    if len(pending) >= 3:
                        emit_pv(pending.pop(0))
                    if companion is not None:
                        companion(ski)
                    yield
                for item in pending:
                    emit_pv(item)

                # softmax normalization: divide by the ones-column sums
                rcp = rpool.tile([1, W], f32, tag="r", bufs=1)
                bc = rpool.tile([64, W], f32, tag="bc", bufs=1)
                dcp = rpool.tile([1, W], f32, tag="rtmp")
                nc.vector.tensor_copy(dcp[:], ctx_ps[DH:DH + 1, :])
                # NOTE: the custom-DVE reciprocal must read SBUF — a PSUM
                # source returns garbage on hardware.  fast variant: ~51 ULP,
                # negligible next to the ~2e-4 float32r matmul rounding
                nc.vector.reciprocal_approx_fast(out=rcp[:], in_=dcp[:])
                if "norm" in debug_dumps:
                    cu = rpool.tile([DH + 1, W], f32, tag="cu")
                    nc.vector.tensor_copy(cu[:], ctx_ps[:])
                    nc.gpsimd.dma_start(dcu[h, j], cu[:])
                    nc.gpsimd.dma_start(dr[h, j:j + 1, :], rcp[0:1, :])
                nc.gpsimd.partition_broadcast(bc[:], rcp[:], channels=64)
                if "norm" in debug_dumps:
                    nc.gpsimd.dma_start(dbc[h, j], bc[:])
                nc.vector.tensor_mul(
                    ctx_sb[po:po + DH, hm, j * W:(j + 1) * W],
                    ctx_ps[0:DH, :],
                    bc[:],
                )

            # ---------------- Phase 1: q/k/v projections -----------------
            qk_cm = tc.tile_pool(name="qkps", bufs=3, space="PSUM")
            qkps = qk_cm.__enter__()
            vps_cm = tc.tile_pool(name="vps", bufs=2, space="PSUM")
            vpool = vps_cm.__enter__()
            with tc.tile_pool(name="xtp", bufs=1) as xpool:
                wq_sb = xpool.tile([128, KD, M], f32r, tag="wq")
                wk_sb = xpool.tile([128, KD, M], f32r, tag="wk")
                wv_sb = xpool.tile([128, KD, M], f32r, tag="wv")
                xt = xpool.tile([128, KD, S], f32r, tag="xt")
                wq_r = wq.rearrange("(k p) m -> p k m", p=128)
                wk_r = wk.rearrange("(k p) m -> p k m", p=128)
                wv_r = wv.rearrange("(k p) m -> p k m", p=128)
                xT_r = xT.rearrange("(k p) s -> p k s", p=128)
                # chunked loads so the projection matmuls can start while
                # later chunks are still in flight; x streams in S-halves so
                # the first half's q/k groups unlock at twice the rate
                nhalf = 2 if (S // 512) >= 4 else 1
                xh = S // nhalf
                for k in range(KD):
                    nc.sync.dma_start(wq_sb[:, k], wq_r[:, k])
                    nc.sync.dma_start(wk_sb[:, k], wk_r[:, k])
                    nc.sync.dma_start(xt[:, k, 0:xh], xT_r[:, k, 0:xh])
                for k in range(KD):
                    if nhalf > 1:
                        nc.sync.dma_start(xt[:, k, xh:S], xT_r[:, k, xh:S])
                for k in range(KD):
                    nc.sync.dma_start(wv_sb[:, k], wv_r[:, k])
                nc.sync.dma_start(wo_sb[:], wo.rearrange("(k p) d -> p k d", p=128))

                nper = (S // 512) // nhalf
                for half in range(nhalf):
                    for m in range(MQ):
                        for n in range(half * nper, (half + 1) * nper):
                            sl = slice(n * 512, n * 512 + 512)
                            psq = qkps.tile([128, 512], f32, tag="psq")
                            psk = qkps.tile([128, 512], f32, tag="psk")
                            for k in range(KD):
                                nc.tensor.matmul(
                                    psq[:],
                                    wq_sb[:, k, m * 128:(m + 1) * 128],
                                    xt[:, k, sl],
                                    start=(k == 0),
                                    stop=(k == KD - 1),
                                )
                            for k in range(KD):
                                nc.tensor.matmul(
                                    psk[:],
                                    wk_sb[:, k, m * 128:(m + 1) * 128],
                                    xt[:, k, sl],
                                    start=(k == 0),
                                    stop=(k == KD - 1),
                                )
                            nc.vector.tensor_copy(qT_sb[:, m, sl], psq[:])
                            nc.vector.tensor_copy(kT_sb[:, m, sl], psk[:])
                for st in range(ST):
                    psv = vpool.tile([128, M], f32, tag="psv", name="psv")
                    for k in range(KD):
                        nc.tensor.matmul(
                            psv,
                            xt[:, k, st * 128:(st + 1) * 128],
                            wv_sb[:, k, :],
                            start=(k == 0),
                            stop=(k == KD - 1),
                        )
                    vdst = v_sb[:, st].rearrange("p (h c) -> p h c", h=HL)[:, :, 0:DH]
                    vsrc = psv[:].rearrange("p (h c) -> p h c", h=HL)
                    nc.vector.tensor_copy(vdst, vsrc)
                    # ones column for the PV denominator trick (in0*0 + 1)
                    ones_v = v_sb[:, st].rearrange("p (h c) -> p h c", h=HL)[:, :, DH]
                    nc.vector.tensor_scalar(
                        ones_v, psv[:, 0:HL], 0.0, 1.0,
                        Alu.mult, Alu.add,
                    )
                vps_cm.__exit__(None, None, None)
                qk_cm.__exit__(None, None, None)

            # ------------- Phase 2: attention ----------------------------
            sc_cm = tc.tile_pool(name="scps", bufs=2, space="PSUM")
            spool = sc_cm.__enter__()
            ctx_cm = tc.tile_pool(name="ctxps", bufs=2, space="PSUM")
            cpool = ctx_cm.__enter__()
            # the last block's sk-steps alternate with out-projection tiles
            # for the s-range whose ctx is already complete (all heads, j<NJ-1)
            ov_split = 0  # out-proj overlap into attention measured slower

            def last_companion(ski):
                if ski % 2 == 0 and ski // 2 < ov_split:
                    emit_outproj_st(ski // 2, spool, "big")

            for j in range(NJ):
                for h in range(HL):
                    last = (h == HL - 1 and j == NJ - 1)
                    comp = last_companion if (last and ov_split) else None
                    for _ in attention_hj(h, j, companion=comp):
                        pass
            ctx_cm.__exit__(None, None, None)
            sc_cm.__exit__(None, None, None)

            # ---------------- Phase 3: output projection -----------------
            with (
                tc.tile_pool(name="ops", bufs=2, space="PSUM") as opool,
                tc.tile_pool(name="ostage", bufs=3) as ostage_p3,
            ):
                ostage[0] = ostage_p3
                for st in range(ov_split, ST):
                    emit_outproj_st(st, opool, "ops")

    nc.compile()
    return nc


def _get_program():
    key = (S, D, HL, DH)
    if key not in _PROGRAM_CACHE:
        _PROGRAM_CACHE[key] = build_program(*key)
    return _PROGRAM_CACHE[key]


def make_in_maps(x, Wq, Wk, Wv, Wo):
    x = np.asarray(x, dtype=np.float32)
    Wq = np.asarray(Wq, dtype=np.float32)
    Wk = np.asarray(Wk, dtype=np.float32)
    Wv = np.asarray(Wv, dtype=np.float32)
    Wo = np.asarray(Wo, dtype=np.float32)
    xTs = [np.ascontiguousarray(x[b].T) for b in range(B)]
    in_maps = []
    for c in range(NCORES):
        b, g = divmod(c, NCORES // B)
        sl = slice(HL * DH * g, HL * DH * (g + 1))
        in_maps.append(
            {
                "xT": xTs[b],
                "wq": np.ascontiguousarray(Wq[sl, :].T),
                "wk": np.ascontiguousarray(Wk[sl, :].T),
                "wv": np.ascontiguousarray(Wv[sl, :].T),
                "wo": np.ascontiguousarray(Wo[:, sl].T),
            }
        )
    return in_maps


def kernel(x, Wq, Wk, Wv, Wo, bo):
    from concourse import bass2jax

    nc = _get_program()
    in_maps = make_in_maps(x, Wq, Wk, Wv, Wo)
    res = bass2jax.run_bass_via_pjrt(nc, in_maps, n_cores=NCORES)
    outs = [res[c]["out"] for c in range(NCORES)]
    gpb = NCORES // B
    o = np.stack([sum(outs[b * gpb + g] for g in range(gpb)) for b in range(B)])
    o = o + np.asarray(bo, dtype=np.float32)[None, None, :]
    return o.astype(np.float32)

